# revision 31
# baseline (speedup 1.0000x reference)
"""ABMIL attention-pooling kernel for 8 TRN2 NeuronCores (Bass/Tile).

Reference computation (per bag b of B=4, N=20000 instances, 1024 feats):
    h   = x @ W_pe + b_pe                    [N, 512]
    A_V = tanh(h @ W_V + b_V)                [N, 128]
    A_U = sigmoid(h @ W_U + b_U)             [N, 128]
    a   = (A_V * A_U) @ w_att + b_att        [N, 1]
    A   = softmax(a, axis=0)
    M   = sum(A * h, axis=0)                 [512]
    out = M @ W_cls + b_cls

Algebraic rewrites (all weight-only precomputation, done host-side):
  * h @ W_V == x @ (W_pe @ W_V): the attention branches never need h.
    With P_V = W_pe@W_V, P_U = W_pe@W_U (both [1024,128]) and
    bias'_V = b_pe@W_V + b_V etc., the gates are rank-256 projections of
    x directly -- the [1024x512] patch embed disappears from the device.
  * logits = (sum_n w_n h_n / s) @ W_cls + b_cls
           = (sum_n w_n (x_n @ P_cls)) / s + (b_pe@W_cls + b_cls),
    with P_cls = W_pe@W_cls [1024, 2] and w_n = exp(a_n), s = sum w_n.
    So the device only needs z = sum_n w_n c_n (c_n = x_n@P_cls, rank 2)
    and s -- never the 512-dim pooled embedding.
  * The 8 c_n matmuls (M=2, 126/128 PE columns idle) are issued as 4
    CONCURRENT column-tiled streams (tile_size 128x32): group j
    accumulates feature chunks {j, j+4} into PSUM partitions {32j,
    32j+1}.  The 4 groups stream different x chunks through different
    col-strips simultaneously, so 8 matmuls cost ~2 matmul spans.  The
    partials stay spread over 4 partition quadrants: ACT/DVE op cost
    depends only on the free dim, so one [98,500] copy / multiply /
    reduce handles all of them and the host sums 8 rows.
  * sigmoid(y) = (1 + tanh(y/2))/2 so tanh/exp/identity are the only ACT
    functions -> one ACT table set, no table swaps.  0.5 factors fold
    into w_att (host) and the U-branch bias.
  * w_att is replicated to 128 columns host-side so the logit matmul
    broadcasts the logit onto all 128 partitions (w16 must align with
    the c partials in all 4 quadrants).

fp8 was evaluated and rejected: quantizing the P_* weights creates an
instance-correlated attention tilt that does NOT average out in the pool
(measured 8.7e-2 rel err vs the 2e-2 gate), and DoubleRow requires both
operands fp8.  f16 everywhere measures ~8.5e-4.

A DVE weighted-x-sum variant (xbar path) was measured at 7.35us/tile of
DVE time (TENSOR_REDUCE is 1x mode, ~780ns per [128,500] reduce) and
DVE-bound 161us total; this all-PE version keeps DVE at ~2.8us/tile.

Sharding: core c -> bag c//2, instance half c%2 (10000 instances each);
the host sums the two partials per bag and applies the constant.

Pipeline (software-pipelined 3 deep, per python iteration t):
    PE : cls(t-2) 8MM-in-2-spans + logit(t-2) + V(t) 8MM + U(t) 8MM
    ACT: gv(t-1), gu(t-1) tanh; w16(t-2) exp; c16(t-2) copy
    DVE: g16(t-1) gate product; zprod(t-2); zred(t-2); sred(t-2)
PE is the bottleneck engine (~5.3us/iter with the measured 262ns/MM
full-weight-reload pacing); ACT ~3.2us, DVE ~2.8us.
"""

import os
import sys

import numpy as np

# Shapes for this problem (hardcoded per the task contract).
B = 4
N = 20000
IN_DIM = 1024
EMB = 512
ATT = 128
NCLS = 2
N_CORES = 8
N_SHARD = (B * N) // N_CORES  # 10000 instances per core
TILE = 500                    # instances per device tile
N_TILES = N_SHARD // TILE     # 20
KC = IN_DIM // 128            # 8 feature chunks
N_GRP = 4                     # concurrent cls col-tile groups
K_PER_GRP = KC // N_GRP       # feature chunks accumulated per group

# packed weight layout (f16, [128, WPACK_COLS]):
#   [0:1024)        P_V     as [ki, k*128+a]
#   [1024:2048)     P_U     as [ki, k*128+a]
#   [2048:2176)     watt128 (0.5*w_att replicated to 128 cols)
#   [2176:2192)     P_cls   as [ki, k*2+c]
W_V_OFF = 0
W_U_OFF = 1024
W_ATT_OFF = 2048
W_CLS_OFF = 2176
WPACK_COLS = W_CLS_OFF + KC * NCLS

# packed bias layout (f32, [128, 3]): [0] bias'_V; [1] 0.5*bias'_U;
# [2] b_att (broadcast all rows)
BPACK_COLS = 3

# out layout (f32, [128, N_TILES, 2]): [:, t, 0] = z partials on rows
# {32j, 32j+1}; [0, t, 1] = s partial.
OUT_SEC = 2
OUT_COLS = N_TILES * OUT_SEC
ZROWS = 32 * (N_GRP - 1) + NCLS   # 98: partition rows spanned by c partials

_cache = {}


def _import_concourse():
    for p in ("/opt/trn_rl_repo", "/root/.axon_site",
              "/root/.axon_site/_ro/trn_rl_repo"):
        if os.path.isdir(p) and p not in sys.path:
            sys.path.append(p)
    import concourse.bass as bass          # noqa: F401
    import concourse.tile as tile          # noqa: F401
    from concourse import mybir            # noqa: F401
    return bass, tile, mybir


def _build_graph():
    bass, tile, mybir = _import_concourse()
    from concourse import bacc
    f16 = mybir.dt.float16
    f32 = mybir.dt.float32
    AF = mybir.ActivationFunctionType
    ALU = mybir.AluOpType

    f8 = mybir.dt.float8e3
    nc = bacc.Bacc("TRN2", target_bir_lowering=False, debug=False,
                   num_devices=N_CORES)

    # x ships tile-major: [tile, ki, k*TILE+inst] so each tile is one DMA
    # with an 8KB contiguous run per partition.  Tiles 0-1 ADDITIONALLY
    # ship as fp8 e3m4 (half the bytes) for their GATE matmuls only: the
    # kernel ramp is gated by the first x DMA's transfer+receipt, and a
    # per-instance fp8 gate error on 2/20 tiles dilutes to ~3e-3 rel on
    # the pooled logits (f16 lhsT x fp8 rhs matmuls run at normal rate;
    # e3m4's 4-bit mantissa halves e4m3's error and x~N(0,1) fits its
    # +-15.5 range).  Their cls/zprod path reads the f16 copies, which
    # arrive a few us later, off the critical ramp.
    xT = nc.declare_dram_parameter("xT", [N_TILES, 128, KC * TILE], f16,
                                   isOutput=False)
    xTq = nc.declare_dram_parameter("xTq", [2, 128, KC * TILE], f8,
                                    isOutput=False)
    wpack = nc.declare_dram_parameter("wpack", [128, WPACK_COLS], f16,
                                      isOutput=False)
    bpack = nc.declare_dram_parameter("bpack", [128, BPACK_COLS], f32,
                                      isOutput=False)
    out = nc.declare_dram_parameter("out", [128, OUT_COLS], f32,
                                    isOutput=True)

    xT_r = xT.rearrange("t p (k i) -> t p k i", k=KC)
    xTq_r = xTq.rearrange("t p (k i) -> t p k i", k=KC)
    out_r = out.rearrange("p (t s) -> p t s", t=N_TILES)

    with tile.TileContext(nc) as tc:
        with (
            tc.tile_pool(name="singles", bufs=1) as singles,
            tc.tile_pool(name="xin", bufs=6) as xin,
            tc.tile_pool(name="gates", bufs=2) as gates,
            tc.tile_pool(name="wexp", bufs=2) as wexp,
            tc.tile_pool(name="acc", bufs=1) as accp,
            tc.tile_pool(name="ps_v", bufs=2, space="PSUM") as ps_v,
            tc.tile_pool(name="ps_u", bufs=2, space="PSUM") as ps_u,
            tc.tile_pool(name="ps_lg", bufs=2, space="PSUM") as ps_lg,
            tc.tile_pool(name="ps_c", bufs=2, space="PSUM") as ps_c,
        ):
            # PE warm-up spin, emitted FIRST: the HAM clock gate keeps the
            # PE at 1.2 GHz until ~3.4us of sustained activity, so a
            # dependency-free matmul spin flips it to 2.4 GHz while the
            # preamble DMAs land.
            warm_sb = singles.tile([128, 512], f16)
            nc.vector.memset(warm_sb, 0.0)
            warm_ps = ps_lg.tile([128, 512], f32, tag="lg")
            for _ in range(12):
                nc.tensor.matmul(warm_ps[:64, :], lhsT=warm_sb[:, 0:64],
                                 rhs=warm_sb, start=True, stop=True)

            # ---- preamble: interleaved weight/x DMA ladder so the first
            # V matmul is gated on ~768KB, not ~2.6MB.  (A finer 7-issue
            # ladder measured ~4us SLOWER: the serialized ~0.65us issue
            # cost per dma_start on the Sync queue dominates.)
            wp = singles.tile([128, WPACK_COLS], f16)
            xt0 = singles.tile([128, KC, TILE], f8)
            nc.sync.dma_start(out=wp[:, 0:W_U_OFF], in_=wpack[:, 0:W_U_OFF])
            nc.sync.dma_start(out=xt0[:, 0:4, :], in_=xTq_r[0, :, 0:4, :])
            nc.sync.dma_start(out=wp[:, W_U_OFF:], in_=wpack[:, W_U_OFF:])
            nc.sync.dma_start(out=xt0[:, 4:8, :], in_=xTq_r[0, :, 4:8, :])
            bias_sb = singles.tile([128, BPACK_COLS], f32)
            nc.sync.dma_start(out=bias_sb, in_=bpack[:, :])
            xt1 = singles.tile([128, KC, TILE], f8)
            nc.sync.dma_start(out=xt1, in_=xTq_r[1])

            pv = wp[:, W_V_OFF:W_U_OFF].rearrange("p (k a) -> p k a", k=KC)
            pu = wp[:, W_U_OFF:W_ATT_OFF].rearrange("p (k a) -> p k a", k=KC)
            watt128 = wp[:, W_ATT_OFF:W_CLS_OFF]
            pcls = wp[:, W_CLS_OFF:WPACK_COLS].rearrange(
                "p (k c) -> p k c", k=KC)

            # ACT-local biases (feed activation bias port)
            bact = singles.tile([128, BPACK_COLS], f32)
            nc.scalar.activation(bact, bias_sb, AF.Identity)

            acc = accp.tile([128, N_TILES, OUT_SEC], f32)

            xts = {}
            xqs = {0: xt0, 1: xt1}
            gvs = {}
            gus = {}
            g16s = {}
            zps = {}

            def front_pe(t):
                """x DMA (t+2) + V/U gate matmuls for tile t.  For t<2
                the gate matmuls read the fp8 preamble tiles; the f16
                copies for the cls/zprod path are DMA'd here (they only
                need to land by iteration t+2)."""
                if t + 2 < N_TILES:
                    xt = xin.tile([128, KC, TILE], f16, tag="xt")
                    nc.sync.dma_start(out=xt, in_=xT_r[t + 2])
                    xts[t + 2] = xt
                if t < 2:
                    xtf = xin.tile([128, KC, TILE], f16, tag="xt")
                    nc.sync.dma_start(out=xtf, in_=xT_r[t])
                    xts[t] = xtf
                xt = xqs[t] if t < 2 else xts[t]
                av = ps_v.tile([128, 512], f32, tag="av")
                au = ps_u.tile([128, 512], f32, tag="au")
                for k in range(KC):
                    nc.tensor.matmul(av[:, :TILE], lhsT=pv[:, k, :],
                                     rhs=xt[:, k, :],
                                     start=(k == 0), stop=(k == KC - 1))
                for k in range(KC):
                    nc.tensor.matmul(au[:, :TILE], lhsT=pu[:, k, :],
                                     rhs=xt[:, k, :],
                                     start=(k == 0), stop=(k == KC - 1))
                return av, au

            def cls_pe(t, c_ps):
                """cls matmuls: 4 concurrent col-tile groups, K_PER_GRP
                sequential accumulations each.  (A col-tiled 4-quadrant
                logit was tried too -- saves one mode switch -- but
                crashed NRT_EXEC_UNIT_UNRECOVERABLE once and measured
                20us SLOWER; the full-array logit stays.)"""
                xt = xts[t]
                for r in range(K_PER_GRP):
                    for j in range(N_GRP):
                        k = j + N_GRP * r
                        p0 = 32 * j
                        nc.tensor.matmul(
                            c_ps[p0:p0 + NCLS, :TILE], lhsT=pcls[:, k, :],
                            rhs=xt[:, k, :], tile_position=(0, p0),
                            start=(r == 0), stop=(r == K_PER_GRP - 1))

            def mid_act(t, av, au):
                """gv = tanh(yV + bV); gu = tanh(yU/2 + bU/2)."""
                gv = gates.tile([128, TILE], f16, tag="gv")
                nc.scalar.activation(gv, av[:, :TILE], AF.Tanh,
                                     bias=bact[:, 0:1])
                gu = gates.tile([128, TILE], f16, tag="gu")
                nc.scalar.activation(gu, au[:, :TILE], AF.Tanh,
                                     bias=bact[:, 1:2], scale=0.5)
                gvs[t], gus[t] = gv, gu

            def back_act(t, lg):
                """w = exp(logit + b_att) on all 128 partitions."""
                w16 = wexp.tile([128, TILE], f16, tag="w16")
                nc.scalar.activation(w16, lg[:, :TILE], AF.Exp,
                                     bias=bact[:, 2:3])
                return w16

            def mid_dve(t):
                """g16 = (gu + 1) * gv  (= 2 * A_V * A_U)."""
                g16 = gates.tile([128, TILE], f16, tag="g16")
                nc.vector.scalar_tensor_tensor(
                    g16, gus[t], 1.0, gvs.pop(t), op0=ALU.add,
                    op1=ALU.mult)
                g16s[t] = g16

            def back_dve(t, w16, c_ps):
                """z/s partials: one FD-bound multiply+reduce covers all
                4 quadrants; garbage rows in between are never read.  The
                multiply reads c_ps straight from PSUM (1x mode) -- this
                is cheaper than an ACT f16 copy plus a 2x multiply."""
                del xts[t]
                zprod = wexp.tile([128, TILE], f16, tag="zprod")
                nc.vector.tensor_mul(zprod[0:ZROWS, :], w16[0:ZROWS, :],
                                     c_ps[0:ZROWS, :TILE])
                nc.vector.reduce_sum(acc[0:ZROWS, t, 0:1], zprod[0:ZROWS, :],
                                     axis=mybir.AxisListType.X)
                nc.vector.reduce_sum(acc[0:1, t, 1:2], w16[0:1, :],
                                     axis=mybir.AxisListType.X)
                zps[t] = zprod

            # NOTE: batching back phases in PAIRS (to halve the tiling-
            # mode switches) measured 19us SLOWER: the denser sustained
            # V/U matmul stream pushed the chip into the P0 power state
            # (PE at 2.0 GHz: MM gap 213 -> 252ns, dur 374 -> 448ns).
            # The per-tile back-block "breathers" keep the PE at 2.4 GHz.
            avaus = {}
            wcs = {}
            for t in range(N_TILES + 1):
                lg = ps_lg.tile([128, 512], f32, tag="lg")
                c_ps = ps_c.tile([128, 512], f32, tag="c")
                if t >= 2:
                    # Observer matmul: a free-dim-1 read of gu(t-2) (long
                    # since finished -- zero stall) advances the PE's view
                    # of the ACT vector clock past every ACT op through
                    # iteration t-1, so the start=True matmuls' WAR legs
                    # (av/au vs tanh reads, lg vs exp read) are
                    # pre-observed and bacc emits no event-semaphore
                    # relays on the ACT queue (measured 2.35us/iter).
                    # Full-K shape keeps it in the cls (128,32) tiling
                    # mode (hidden in the mode-switch gap); its dst is
                    # overwritten by the logit matmul.
                    guo = gus.pop(t - 2)
                    nc.tensor.matmul(lg[0:1, 0:1], lhsT=guo[:, 0:1],
                                     rhs=guo[:, 0:1], tile_position=(0, 0),
                                     start=True, stop=True)
                    cls_pe(t - 2, c_ps)
                    nc.tensor.matmul(lg[:, :TILE], lhsT=watt128,
                                     rhs=g16s.pop(t - 2),
                                     start=True, stop=True)
                if t < N_TILES:
                    avaus[t] = front_pe(t)
                if 1 <= t <= N_TILES:
                    av, au = avaus.pop(t - 1)
                    mid_act(t - 1, av, au)
                if t >= 2:
                    wcs[t - 2] = back_act(t - 2, lg)
                if 1 <= t <= N_TILES:
                    mid_dve(t - 1)
                if t >= 2:
                    back_dve(t - 2, wcs.pop(t - 2), c_ps)
                if t - 2 == N_TILES - 3:
                    # early writeback of tiles 0..N_TILES-3: hides the bulk
                    # of the output DMA under the last two tiles' compute.
                    nc.sync.dma_start(out=out_r[:, 0:N_TILES - 2, :],
                                      in_=acc[:, 0:N_TILES - 2, :])

            # epilogue: the last tile's back phase issues right behind
            # tile N_TILES-2's (its g16 lands mid-iteration), compressing
            # the pipeline drain by roughly one iteration.
            tl = N_TILES - 1
            lg = ps_lg.tile([128, 512], f32, tag="lg")
            c_ps = ps_c.tile([128, 512], f32, tag="c")
            cls_pe(tl, c_ps)
            nc.tensor.matmul(lg[:, :TILE], lhsT=watt128, rhs=g16s.pop(tl),
                             start=True, stop=True)
            wcs[tl] = back_act(tl, lg)
            back_dve(tl, wcs.pop(tl), c_ps)

            # tail writeback: the last two tiles' partial columns
            nc.sync.dma_start(out=out_r[:, N_TILES - 2:N_TILES, :],
                              in_=acc[:, N_TILES - 2:N_TILES, :])

            # tail spin: dependency-free matmuls hold the HAM clock at
            # 2.4 GHz while the last tiles' exp/pool chain drains on
            # ACT/DVE and the TileContext teardown barrier completes.
            warm_ps2 = ps_v.tile([128, 512], f32, tag="av")
            for _ in range(40):
                nc.tensor.matmul(warm_ps2[:64, :], lhsT=warm_sb[:, 0:64],
                                 rhs=warm_sb, start=True, stop=True)

    nc.compile()
    return nc


def _prep_in_maps(x, W_pe, b_pe, W_V, b_V, W_U, b_U, w_att, b_att):
    f16 = np.float16
    f32 = np.float32
    f64 = np.float64

    W_pe64 = W_pe.astype(f64)
    P_V = (W_pe64 @ W_V.astype(f64)).astype(f32)     # [1024, 128]
    P_U = (W_pe64 @ W_U.astype(f64)).astype(f32)

    wpack = np.empty((128, WPACK_COLS), dtype=f16)
    wpack[:, W_V_OFF:W_U_OFF] = (
        P_V.reshape(KC, 128, ATT).transpose(1, 0, 2).reshape(128, KC * ATT))
    wpack[:, W_U_OFF:W_ATT_OFF] = (
        P_U.reshape(KC, 128, ATT).transpose(1, 0, 2).reshape(128, KC * ATT))
    # kernel computes G' = 2*A_V*A_U; 0.5*w_att undoes the factor of 2.
    wpack[:, W_ATT_OFF:W_CLS_OFF] = np.repeat(
        0.5 * w_att.astype(f32), 128, axis=1)
    P_cls = _cache["P_cls64"].astype(f32)             # [1024, 2]
    wpack[:, W_CLS_OFF:WPACK_COLS] = (
        P_cls.reshape(KC, 128, NCLS).transpose(1, 0, 2)
        .reshape(128, KC * NCLS))

    bias_V = (b_pe.astype(f64) @ W_V.astype(f64) + b_V).astype(f32)
    bias_U = (b_pe.astype(f64) @ W_U.astype(f64) + b_U).astype(f32)
    bpack = np.empty((128, BPACK_COLS), dtype=f32)
    bpack[:, 0] = bias_V
    bpack[:, 1] = 0.5 * bias_U
    bpack[:, 2] = b_att[0]

    import ml_dtypes
    f8 = ml_dtypes.float8_e3m4

    shared = {"wpack": wpack, "bpack": bpack}
    in_maps = []
    half = N // 2
    for c in range(N_CORES):
        bag, hi = divmod(c, 2)
        xs = x[bag, hi * half:(hi + 1) * half, :]
        xt_tiles = np.ascontiguousarray(
            xs.T.astype(f16).reshape(KC, 128, N_TILES, TILE)
            .transpose(2, 1, 0, 3).reshape(N_TILES, 128, KC * TILE))
        in_maps.append({"xT": xt_tiles,
                        "xTq": xt_tiles[:2].astype(f8), **shared})
    return in_maps


def _run(inputs, trace=False, tmpdir=None):
    _import_concourse()
    from concourse.bass_utils import run_bass_kernel_spmd

    if "nc" not in _cache:
        _cache["nc"] = _build_graph()
    nc = _cache["nc"]

    W_pe64 = np.asarray(inputs["W_pe"], dtype=np.float64)
    _cache["P_cls64"] = W_pe64 @ np.asarray(inputs["W_cls"], np.float64)

    in_maps = _prep_in_maps(
        inputs["x"], inputs["W_pe"], inputs["b_pe"], inputs["W_V"],
        inputs["b_V"], inputs["W_U"], inputs["b_U"], inputs["w_att"],
        inputs["b_att"])

    res = run_bass_kernel_spmd(
        nc, in_maps, core_ids=list(range(N_CORES)),
        trace=trace, tmpdir=tmpdir)

    b_pe = np.asarray(inputs["b_pe"], np.float64)
    W_cls = np.asarray(inputs["W_cls"], np.float64)
    b_cls = np.asarray(inputs["b_cls"], np.float64)
    cls_const = b_pe @ W_cls + b_cls

    logits = np.zeros((B, NCLS), dtype=np.float32)
    for bag in range(B):
        z = np.zeros(NCLS, dtype=np.float64)
        s = 0.0
        for hi in range(2):
            o = res.results[2 * bag + hi]["out"].astype(np.float64)
            o = o.reshape(128, N_TILES, OUT_SEC)
            for j in range(N_GRP):
                z += o[32 * j:32 * j + NCLS, :, 0].sum(axis=1)
            s += o[0, :, 1].sum()
        logits[bag] = (z / s + cls_const).astype(np.float32)
    return logits, res


def kernel(**inputs):
    inputs = {k: np.asarray(v) for k, v in inputs.items()}
    logits, _ = _run(inputs, trace=False)
    return logits
